# revision 48
# baseline (speedup 1.0000x reference)
"""Multi-head attention (B=4, T=S=2048, H=1024, 16 heads x D=64) on 8 TRN2 cores.

Sharding: 2D mesh of batch(4) x head-group(2). Core c = b*2 + g computes, for
its batch b and its 8 heads (ND slice g*512:(g+1)*512):
  - q/k/v projections (bf16 matmuls, fp32 PSUM accumulate)
  - attention in transposed [S, T] orientation: scoresT = kT.T @ qT chunks,
    exp on ScalarE (1/sqrt(D) folded into the activation scale), softmax
    denominator via a ones-column appended to v in the AV matmul
  - partial output projection out_part = ao @ Wo_g.T  ([T, H], fp32)
Host sums the two head-group partials per batch and adds bo.

Schedule (HW-profiled on the axon-tunneled TRN2 cores):
  - ScalarE exp (~33M elements/core, ~0.99us per [128,1024] chunk) and
    TensorE (~1550 matmuls) are both near-saturated; projection /
    output-projection matmuls interleave between attention s-chunks via a
    deadline-forced background queue (force() guarantees write-before-read
    emission order for Tile's dependency tracking).
  - Per chunk, the previous chunk's AV matmuls and bg filler are emitted
    BEFORE the score matmuls: PE executes its stream in order, so filler
    placed after a score blocked on the sc-slot WAR would head-of-line
    stall (measured ~20us).
  - Softmax normalization: the DVE iterative-divide reciprocal is serial
    along the free dim (~1.5us for [1,512], ~50us/exec exposed); the 512
    denominators are DMA-scattered to [128,4], reciprocal'd partition-
    parallel (~30ns), gathered back (measured: whole normalize now hides
    under the attention stream).
  - Input DMAs are batched 1MB/transfer (the ~2us fixed DMA cost x36
    small transfers dominated the lead-in), tiny bias loads ride the
    gpsimd SWDGE queue, queues ordered by first-use time.
  - Last pair's last t-block does per-128-col normalize + output
    projection to shorten the serial tail; in-schedule warmup matmuls
    bridge each exec's input-DMA wait to keep the PE HAM clock gate open.

All matmul inputs bf16: rel err vs fp32 reference ~4e-3. q/k/v biases are
applied in-kernel (zero for this problem, but supported); bo added on host.

_build(loop=N) wraps the whole per-exec schedule (input DMA + compute +
stores) in a hardware For_i loop with a full barrier per iteration —
test.py times (t[loop=R]-t[loop=1])/(R-1) to amortize the noisy multi-ms
host/tunnel dispatch overhead out of the measurement. mode="proj"/"attn*"
builds are single-purpose probes used during optimization.
"""

from collections import deque

import numpy as np
import ml_dtypes

import concourse.bacc as bacc
import concourse.mybir as mybir
import concourse.tile as tile
from concourse.bass_utils import run_bass_kernel_spmd

B, T, H = 4, 2048, 1024
N_HEADS, D = 16, 64
GROUPS = 2
HEADS_PER_GROUP = N_HEADS // GROUPS          # 8
NDG = HEADS_PER_GROUP * D                    # 512
SCALE = 1.0 / float(D) ** 0.5
N_CORES = 8
TB = 512                                     # attention T-block

bf16 = mybir.dt.bfloat16
f32 = mybir.dt.float32
EXP = mybir.ActivationFunctionType.Exp
MULT = mybir.AluOpType.mult
ADD = mybir.AluOpType.add

_CACHED_NC = None
_CACHED_ZB = None


def _build(repeat=1, loop=0, inputs_in_loop=True, mode="full",
           zero_bias=False, burst=2):
    """loop=N>0 wraps the schedule in a hardware For_i loop executing the
    full schedule N times (used for steady-state HW timing); loop=0 emits
    the schedule `repeat` times inline (normal path). inputs_in_loop=False
    hoists the input DMAs out of the loop body (diagnostic: isolates
    compute-only steady state). mode: "full" | "proj" (projections +
    output projection only) | "attn" (attention core only) — HW probes."""
    nc = bacc.Bacc("TRN2", target_bir_lowering=False, debug=False)

    xq_d = nc.dram_tensor("xqT", (H, T), bf16, kind="ExternalInput")
    xv_d = nc.dram_tensor("xvT", (H, T), bf16, kind="ExternalInput")
    wq_d = nc.dram_tensor("wqT", (H, NDG), bf16, kind="ExternalInput")
    wk_d = nc.dram_tensor("wkT", (H, NDG), bf16, kind="ExternalInput")
    wv_d = nc.dram_tensor("wvT", (H, NDG), bf16, kind="ExternalInput")
    wo_d = nc.dram_tensor("woT", (NDG, H), bf16, kind="ExternalInput")
    bq_d = nc.dram_tensor("bq", (NDG,), f32, kind="ExternalInput")
    bk_d = nc.dram_tensor("bk", (NDG,), f32, kind="ExternalInput")
    bv_d = nc.dram_tensor("bv", (NDG,), f32, kind="ExternalInput")
    out_d = nc.dram_tensor("outp", (T, H), f32, kind="ExternalOutput")

    with tile.TileContext(nc) as tc:
        with tc.tile_pool(name="w", bufs=1) as wpool, \
             tc.tile_pool(name="data", bufs=1) as dpool, \
             tc.tile_pool(name="exps", bufs=4) as epool, \
             tc.tile_pool(name="norm", bufs=2) as npool, \
             tc.tile_pool(name="stage", bufs=3) as spool, \
             tc.tile_pool(name="ps_sc", bufs=2, space="PSUM") as ps_sc, \
             tc.tile_pool(name="ps_av", bufs=1, space="PSUM") as ps_av, \
             tc.tile_pool(name="ps_pj", bufs=2, space="PSUM") as ps_pj:

            wq_t = wpool.tile([128, 8, NDG], bf16)
            wk_t = wpool.tile([128, 8, NDG], bf16)
            wv_t = wpool.tile([128, 8, NDG], bf16)
            wo_t = wpool.tile([128, 4, H], bf16)
            bq_t = wpool.tile([128, 4], f32)
            bk_t = wpool.tile([128, 4], f32)
            bv_row = wpool.tile([1, NDG], f32)
            bv_bc = wpool.tile([128, NDG], f32)

            xq_t = dpool.tile([128, 8, T], bf16)
            xv_t = dpool.tile([128, 8, T], bf16)
            qT_t = dpool.tile([128, 4, T], bf16)
            kT_t = dpool.tile([128, 4, T], bf16)
            v_t = dpool.tile([128, 16, HEADS_PER_GROUP, D + 1], bf16)
            ao_t = dpool.tile([128, 4, T], bf16)

            xv_r = xv_d.rearrange("(c p) t -> p c t", p=128)
            xq_r = xq_d.rearrange("(c p) t -> p c t", p=128)

            # one-time setup (outside the timing loop): ones column for the
            # softmax-denominator trick, PE warmup fodder
            warm = wpool.tile([128, 512], bf16)
            nc.vector.memset(warm[:], 0.0)
            nc.vector.memset(v_t[:, :, :, D], 1.0)
            exd = wpool.tile([1, 16], bf16)
            exst = None
            if mode == "full_nco":
                nc.vector.memset(kT_t[:], 0.0)
                nc.vector.memset(qT_t[:], 0.0)
                nc.vector.memset(v_t[:, :, :, 0:D], 0.0)
                nc.vector.memset(ao_t[:], 0.0)
            if mode.startswith("attn"):
                nc.vector.memset(kT_t[:], 0.0)
                nc.vector.memset(qT_t[:], 0.0)
                nc.vector.memset(v_t[:, :, :, 0:D], 0.0)
                exst = wpool.tile([128, 2 * TB], bf16)
                nc.vector.memset(exst[:], 0.001)
            elif mode == "proj":
                nc.vector.memset(ao_t[:], 0.0)

            def emit_inputs():
                """Batched input DMAs: one transfer per 1MB x-chunk instead
                of 8x128KB — the ~2us fixed DMA cost dominated the lead-in.
                Tiny bias loads go on the gpsimd SWDGE queue so they don't
                delay the critical wk/xv_c0 transfers; queues are ordered by
                first-use time (xv chunks feed v-projections early, xq_c1+
                aren't needed until t-block 1)."""
                # ACT table preload off the critical path
                nc.scalar.activation(exd[:], warm[0:1, 0:16], EXP, scale=SCALE)
                nc.gpsimd.dma_start(bq_t[:], bq_d.rearrange("(c p) -> p c", p=128))
                nc.gpsimd.dma_start(bk_t[:], bk_d.rearrange("(c p) -> p c", p=128))
                nc.gpsimd.dma_start(bv_row[:], bv_d[None, :])
                nc.sync.dma_start(wk_t[:], wk_d.rearrange("(c p) n -> p c n", p=128))
                nc.sync.dma_start(xv_t[:, :, 0:512], xv_r[:, :, 0:512])
                nc.scalar.dma_start(wq_t[:], wq_d.rearrange("(c p) n -> p c n", p=128))
                nc.scalar.dma_start(xq_t[:, :, 0:512], xq_r[:, :, 0:512])
                nc.sync.dma_start(wv_t[:], wv_d.rearrange("(c p) n -> p c n", p=128))
                nc.sync.dma_start(xv_t[:, :, 512:1024], xv_r[:, :, 512:1024])
                nc.scalar.dma_start(xv_t[:, :, 1024:1536], xv_r[:, :, 1024:1536])
                nc.sync.dma_start(xv_t[:, :, 1536:2048], xv_r[:, :, 1536:2048])
                for t4 in range(1, 4):
                    nc.scalar.dma_start(xq_t[:, :, t4 * 512:(t4 + 1) * 512],
                                        xq_r[:, :, t4 * 512:(t4 + 1) * 512])
                nc.scalar.dma_start(wo_t[:], wo_d.rearrange("(c p) h -> p c h", p=128))
                nc.gpsimd.partition_broadcast(bv_bc[:], bv_row[0:1, :])

            # PE warmup: spins the HAM clock gate up and bridges the input-DMA
            # latency so the PE doesn't idle into a MID window before k0/q0
            wps = ps_pj.tile([128, 512], f32, tag="pj", name="wps")
            for _ in range(20):
                nc.tensor.matmul(wps[:], warm[:, 0:128], warm[:],
                                 start=True, stop=True)

            # ---- background-emission machinery (PE filler work) ----
            # queue of (key, generator); drain(n) steps n matmuls; force(key)
            # drains until the named generator has fully emitted (hard
            # deadline before emitting a consumer of its output)
            bg = deque()
            bg_done = set()

            def drain(n):
                while n > 0 and bg:
                    try:
                        next(bg[0][1])
                        n -= 1
                    except StopIteration:
                        bg_done.add(bg[0][0])
                        bg.popleft()

            def force(key):
                while bg and key not in bg_done:
                    drain(64)

            def drain_all():
                while bg:
                    drain(64)

            nco = mode == "full_nco"   # probe: matmuls without copy-outs

            def gen_proj_qk(dst_t, src_t, w_t, b_t, ndc, t4s=range(4),
                            act_out=False):
                for t4 in t4s:
                    ps = ps_pj.tile([128, 512], f32, tag="pj")
                    for h in range(8):
                        nc.tensor.matmul(
                            ps[:],
                            w_t[:, h, ndc * 128:(ndc + 1) * 128],
                            src_t[:, h, t4 * 512:(t4 + 1) * 512],
                            start=(h == 0), stop=(h == 7),
                        )
                        yield
                    if nco:
                        continue
                    dst = dst_t[:, ndc, t4 * 512:(t4 + 1) * 512]
                    if act_out and zero_bias:
                        # pair-0 window: ScalarE is idle (PE-bound phase) and
                        # is closer to PSUM; frees the DVE and decouples the
                        # pj-slot rotation from the DVE queue
                        nc.scalar.copy(dst, ps[:])
                    else:
                        nc.vector.tensor_tensor(
                            dst, ps[:],
                            b_t[:, ndc, None].to_broadcast((128, 512)), ADD)

            def gen_proj_v(t16s=range(16), act_out=False):
                for t16 in t16s:
                    ps = ps_pj.tile([128, 512], f32, tag="pj")
                    for h in range(8):
                        nc.tensor.matmul(
                            ps[:],
                            xv_t[:, h, t16 * 128:(t16 + 1) * 128],
                            wv_t[:, h, :],
                            start=(h == 0), stop=(h == 7),
                        )
                        yield
                    if nco:
                        continue
                    if act_out and zero_bias:
                        nc.scalar.copy(
                            v_t[:, t16, :, 0:D],
                            ps[:].rearrange("p (hh d) -> p hh d", d=D))
                    else:
                        nc.vector.tensor_tensor(
                            v_t[:, t16, :, 0:D],
                            ps[:].rearrange("p (hh d) -> p hh d", d=D),
                            bv_bc[:].rearrange("p (hh d) -> p hh d", d=D), ADD)

            def gen_oproj(trange):
                for t16 in trange:
                    for hh in range(2):
                        ps = ps_pj.tile([128, 512], f32, tag="pj")
                        for nd in range(4):
                            nc.tensor.matmul(
                                ps[:],
                                ao_t[:, nd, t16 * 128:(t16 + 1) * 128],
                                wo_t[:, nd, hh * 512:(hh + 1) * 512],
                                start=(nd == 0), stop=(nd == 3),
                            )
                            yield
                        if nco:
                            continue
                        st = spool.tile([128, 512], f32, tag="st")
                        nc.vector.tensor_copy(st[:], ps[:])
                        nc.sync.dma_start(
                            out_d[t16 * 128:(t16 + 1) * 128,
                                  hh * 512:(hh + 1) * 512], st[:])

            def attn_pair(p, first=False, post_tb=None, drain_n=3,
                          fine_tail=False, no_exp=False, no_av=False,
                          no_norm=False, norm_upto=-1, drain_skip=0):
                """Heads 2p (partitions 0:64) and 2p+1 (64:128) of chunk p,
                processed together: their score matmuls land in different PE
                row groups and run concurrently; one exp instruction covers
                both heads' [128, 512] score chunks.

                first=True: k/q/v tiles for pair 0 are still being produced
                by bg generators — force() their emission right before the
                first consumer so Tile sees the write-before-read order."""
                for tb in range(T // TB):
                    t0 = tb * TB
                    if first and tb > 0:
                        force(("q", 0, tb))
                    avAB = ps_av.tile([128, 2 * TB], f32, tag="av",
                                      name="avAB")
                    avA = avAB[:, 0:TB]
                    avB = avAB[:, TB:2 * TB]

                    def av_mms(s, ex):
                        if first and tb == 0:
                            force(("v", s))
                        for i, av in ((0, avA), (1, avB)):
                            nc.tensor.matmul(
                                av[0:D + 1, :],
                                v_t[:, s, 2 * p + i, :],
                                ex[:, i * TB:(i + 1) * TB],
                                start=(s == 0), stop=(s == 15),
                            )

                    pending = None
                    for s in range(16):
                        if first and tb == 0 and s % 4 == 0 and s > 0:
                            force(("k", 0, s // 4))
                        # emit the previous chunk's AV matmuls and the bg
                        # filler BEFORE this chunk's score matmuls: the score
                        # stalls on the sc-slot WAR (exp two chunks back), and
                        # PE executes its stream in order — filler placed after
                        # a stalled score would head-of-line block.
                        if not no_av and pending is not None:
                            av_mms(*pending)
                        if s >= drain_skip:
                            if burst:
                                # whole projection groups every `burst`
                                # chunks: fewer weight-set switches in the
                                # PE stream
                                if s % burst == drain_skip % burst:
                                    drain(burst * drain_n)
                            else:
                                drain(drain_n)
                        sc = ps_sc.tile([128, 2 * TB], f32, tag="sc")
                        for i, off in ((0, 0), (1, 64)):
                            nc.tensor.matmul(
                                sc[:, i * TB:(i + 1) * TB],
                                kT_t[off:off + 64, p, s * 128:(s + 1) * 128],
                                qT_t[off:off + 64, p, t0:t0 + TB],
                                start=True, stop=True,
                            )
                        if no_exp:
                            ex = exst  # static tile, probe only
                        else:
                            ex = epool.tile([128, 2 * TB], bf16, tag="exp")
                            nc.scalar.activation(ex[:], sc[:], EXP, scale=SCALE)
                        if not no_av:
                            pending = (s, ex)
                    if not no_av:
                        av_mms(*pending)
                    if no_av or no_norm:
                        continue
                    if norm_upto >= 0:
                        # probe: emit only a prefix of the norm chain
                        for i, av in ((0, avA), (1, avB)):
                            off = 64 * i
                            avs = npool.tile([D + 1, TB], f32, tag="avs")
                            nc.vector.tensor_copy(avs[0:D + 1, :],
                                                  av[0:D + 1, :])
                            if norm_upto >= 1:
                                recip = npool.tile([1, TB], f32, tag="recip")
                                nc.vector.reciprocal(recip[:], avs[D:D + 1, :])
                            if norm_upto >= 2:
                                bc = npool.tile([64, TB], f32, tag="bc")
                                nc.gpsimd.partition_broadcast(bc[:],
                                                              recip[0:1, :])
                        continue
                    if fine_tail and tb == 3:
                        # last t-block of the last pair: normalize in 128-col
                        # pieces and chain each piece's output projection
                        # immediately, shortening the serial kernel tail
                        drain_all()
                        bcs = []
                        for i, av in ((0, avA), (1, avB)):
                            avs = npool.tile([D + 1, TB], f32, tag="avs")
                            nc.vector.tensor_copy(avs[0:D + 1, :],
                                                  av[0:D + 1, :])
                            dt = npool.tile([128, 4], f32, tag="dt")
                            nc.scalar.dma_start(dt[:, :], avs[D:D + 1, :])
                            rt = npool.tile([128, 4], f32, tag="rt")
                            nc.vector.reciprocal(rt[:], dt[:])
                            recip = npool.tile([1, TB], f32, tag="recip")
                            nc.scalar.dma_start(recip[0:1, :], rt[:, :])
                            bc = npool.tile([64, TB], f32, tag="bc")
                            nc.gpsimd.partition_broadcast(bc[:], recip[0:1, :])
                            bcs.append((64 * i, avs, bc))
                        # per-128-col multiply + immediate output projection:
                        # shortens the serial tail after the last AV matmul
                        for q in range(4):
                            cq = slice(q * 128, (q + 1) * 128)
                            cs = slice(t0 + q * 128, t0 + (q + 1) * 128)
                            for off, avs, bc in bcs:
                                nc.vector.tensor_tensor(
                                    ao_t[off:off + 64, p, cs],
                                    avs[0:D, cq], bc[:, cq], MULT)
                            for _ in gen_oproj([12 + q]):
                                pass
                        continue
                    for i, av in ((0, avA), (1, avB)):
                        off = 64 * i
                        avs = npool.tile([D + 1, TB], f32, tag="avs")
                        nc.vector.tensor_copy(avs[0:D + 1, :], av[0:D + 1, :])
                        # reciprocal of the 512 softmax denominators: the DVE
                        # iterative divide is serial along the free dim, so a
                        # [1,512] recip costs ~1.5us. DMA-scatter the row to
                        # [128,4] (partition-parallel recip, ~30ns), gather
                        # back. The tiny DMAs ride the otherwise-idle scalar
                        # HWDGE queue.
                        dt = npool.tile([128, 4], f32, tag="dt")
                        nc.scalar.dma_start(dt[:, :], avs[D:D + 1, :])
                        rt = npool.tile([128, 4], f32, tag="rt")
                        nc.vector.reciprocal(rt[:], dt[:])
                        recip = npool.tile([1, TB], f32, tag="recip")
                        nc.scalar.dma_start(recip[0:1, :], rt[:, :])
                        bc = npool.tile([64, TB], f32, tag="bc")
                        nc.gpsimd.partition_broadcast(bc[:], recip[0:1, :])
                        nc.vector.tensor_tensor(
                            ao_t[off:off + 64, p, t0:t0 + TB],
                            avs[0:D, :], bc[:], MULT)
                    if post_tb is not None:
                        post_tb(tb)

            def emit_schedule_proj():
                # probe: q/k/v projections + output projection, no attention
                emit_inputs()
                for ndc in range(4):
                    for _ in gen_proj_qk(kT_t, xv_t, wk_t, bk_t, ndc):
                        pass
                    for _ in gen_proj_qk(qT_t, xq_t, wq_t, bq_t, ndc):
                        pass
                for _ in gen_proj_v():
                    pass
                for _ in gen_oproj(range(16)):
                    pass

            def emit_schedule_attn():
                # probe: attention core only (qT/kT/v memset once outside)
                kw = {}
                if mode == "attn_noexp":
                    kw = dict(no_exp=True)
                elif mode == "attn_noav":
                    kw = dict(no_av=True)
                elif mode == "attn_nonorm":
                    kw = dict(no_norm=True)
                elif mode.startswith("attn_n"):
                    kw = dict(norm_upto=int(mode[-1]))
                for p in range(4):
                    attn_pair(p, **kw)

            # ---- emission schedule ----
            def emit_schedule():
              if mode == "proj":
                  emit_schedule_proj()
                  return
              if mode.startswith("attn"):
                  emit_schedule_attn()
                  return
              if inputs_in_loop:
                  emit_inputs()
              # keep the PE busy (and the HAM clock gate open) through the
              # input-DMA wait at the start of each execution: junk matmuls
              # into an sc-pool slot, no data dependencies
              wsc = ps_sc.tile([128, 2 * TB], f32, tag="sc", name="wsc")
              for _ in range(16):
                  nc.tensor.matmul(wsc[:, 0:TB], warm[:, 0:128], warm[:],
                                   start=True, stop=True)
              # minimal lead-in: k0/q0 for s,t cols 0:512; attention starts
              # immediately after, remaining k0/q0/v arrive via the bg queue
              # ordered by first-use time with force() deadlines
              for _ in gen_proj_qk(kT_t, xv_t, wk_t, bk_t, 0, [0],
                                   act_out=True):
                pass
              for _ in gen_proj_qk(qT_t, xq_t, wq_t, bq_t, 0, [0],
                                   act_out=True):
                pass

              def qk(dst, src, w, b, tag, ndc, t4, act_out=False):
                  return ((tag, ndc, t4),
                          gen_proj_qk(dst, src, w, b, ndc, [t4],
                                      act_out=act_out))

              def kg(ndc, t4, act_out=False):
                  return qk(kT_t, xv_t, wk_t, bk_t, "k", ndc, t4, act_out)

              def qg(ndc, t4, act_out=False):
                  return qk(qT_t, xq_t, wq_t, bq_t, "q", ndc, t4, act_out)

              def vg(t16):
                  return (("v", t16), gen_proj_v([t16], act_out=True))

              # need-ordered: v_s needed at AV(s) (chunk s+1); k0_g at chunk
              # 4g; q0_tb at t-block tb (chunk 16*tb). Copy-outs drained in
              # pair 0's PE-bound window go to the then-idle ScalarE.
              bg.extend([
                  vg(0), vg(1), vg(2), vg(3), kg(0, 1, True), vg(4), vg(5),
                  vg(6), kg(0, 2, True), vg(7), vg(8), vg(9), vg(10),
                  kg(0, 3, True), vg(11), vg(12), vg(13), vg(14), vg(15),
                  qg(0, 1, True), qg(0, 2, True), qg(0, 3, True),
                  kg(1, 0), kg(1, 1), kg(1, 2), kg(1, 3),
                  qg(1, 0), qg(1, 1), qg(1, 2), qg(1, 3),
              ])
              attn_pair(0, first=True, drain_n=5)
              drain_all()
              for p in range(1, 4):
                if p < 3:
                    for t4 in range(4):
                        bg.append(kg(p + 1, t4))
                    for t4 in range(4):
                        bg.append(qg(p + 1, t4))
                last = (p == 3)

                def last_post(tb):
                    # ao rows tb*512..+512 complete for all heads once the
                    # last pair finishes this t-block
                    if tb < 3:
                        bg.append((("o", tb), gen_oproj(range(4 * tb, 4 * tb + 4))))

                attn_pair(p, post_tb=last_post if last else None,
                          fine_tail=last, drain_skip=2 if last else 0)
                drain_all()

            if loop > 0:
                if not inputs_in_loop:
                    emit_inputs()
                with tc.For_i(0, loop, 1):
                    emit_schedule()
            else:
                for _rep in range(repeat):
                    emit_schedule()

    nc.compile()
    return nc


def kernel(**inputs):
    global _CACHED_NC
    query = np.asarray(inputs["query"], dtype=np.float32)
    value = np.asarray(inputs["value"], dtype=np.float32)
    Wq = np.asarray(inputs["Wq"], dtype=np.float32)
    Wk = np.asarray(inputs["Wk"], dtype=np.float32)
    Wv = np.asarray(inputs["Wv"], dtype=np.float32)
    Wo = np.asarray(inputs["Wo"], dtype=np.float32)
    bq = np.asarray(inputs["bq"], dtype=np.float32)
    bk = np.asarray(inputs["bk"], dtype=np.float32)
    bv = np.asarray(inputs["bv"], dtype=np.float32)
    bo = np.asarray(inputs["bo"], dtype=np.float32)

    # zero_bias=True (pair-0 copy-outs on ScalarE) was tried and measured
    # ~17us slower on HW — ACT head-of-line on the copies outweighs the
    # DVE decoupling. Keep the DVE bias-add path.
    if _CACHED_NC is None:
        _CACHED_NC = _build(zero_bias=False)
    nc = _CACHED_NC

    bf = ml_dtypes.bfloat16
    in_maps = []
    for c in range(N_CORES):
        b, g = c // 2, c % 2
        sl = slice(g * NDG, (g + 1) * NDG)
        in_maps.append({
            "xqT": np.ascontiguousarray(query[b].T).astype(bf),
            "xvT": np.ascontiguousarray(value[b].T).astype(bf),
            "wqT": np.ascontiguousarray(Wq[sl].T).astype(bf),
            "wkT": np.ascontiguousarray(Wk[sl].T).astype(bf),
            "wvT": np.ascontiguousarray(Wv[sl].T).astype(bf),
            "woT": np.ascontiguousarray(Wo[:, sl].T).astype(bf),
            "bq": np.ascontiguousarray(bq[sl]),
            "bk": np.ascontiguousarray(bk[sl]),
            "bv": np.ascontiguousarray(bv[sl]),
        })

    res = run_bass_kernel_spmd(nc, in_maps, core_ids=list(range(N_CORES)))

    out = np.zeros((B, T, H), dtype=np.float32)
    for c in range(N_CORES):
        out[c // 2] += res.results[c]["outp"]
    out += bo
    return out

